# revision 1
# baseline (speedup 1.0000x reference)
"""Multi-head differential attention (full-width) on 8 Trainium2 NeuronCores.

Contract: kernel(**inputs) takes the FULL inputs of reference.setup_inputs()
and returns the FULL [8, 512, 8192] float32 output.

Strategy: pure data-parallel over batch — one batch element per NeuronCore.
Per core, a single fused Bass/Tile kernel computes:
  - qT/kT projections o-block by o-block (float32r matmuls at full PE rate),
    with the score matmuls fused into the same loop so scores finish when the
    projections do (no qT/kT ever stored: only a rotating 2-tile window),
  - softmax via ScalarE Exp with accum_out row-sums (scores are bounded ~±25,
    so no max-subtraction is needed in fp32),
  - differential combine P = e1/d1 - lam ⊙ e2/d2 on VectorE,
  - PE-transpose of P, then a fused v-projection + attention-V matmul loop
    that streams Wv and writes each 512-column output tile as it completes.

Weights are repacked host-side (pure layout permutation + no dtype change) so
every DMA lands with long contiguous per-partition runs.
"""
import ml_dtypes
import numpy as np
from contextlib import ExitStack

import concourse.bass as bass
import concourse.mybir as mybir
import concourse.tile as tile
from concourse.bass_utils import run_bass_kernel_spmd
from concourse.masks import make_identity

F32 = mybir.dt.float32
F32R = mybir.dt.float32r
BF16 = mybir.dt.bfloat16
P = 128
B = 8
S = 512          # sequence length (= d_head for the lambda broadcast)
DM = 4096        # model dim (projection contraction)
DH = 4096        # width of each q/k half (attention contraction)
D2 = 2 * DH      # projection output width
NQB = S // P     # 4 query blocks
NT = DM // P     # 32 contraction tiles
NOB = D2 // P    # 64 projection o-blocks
HOB = DH // P    # 32 o-blocks per half
NODT = D2 // 512  # 16 output column tiles
WV_CH = 8        # dq-tiles per streamed Wv chunk
NCH = NT // WV_CH
SCALE = float(1.0 / np.sqrt(512.0))

MAX_WAITS = 1  # this walrus build allows one sync-wait per instruction


def _split_sync_waits(nc):
    """Move excess per-instruction sync waits onto preceding no-ops (same
    engine, program order) — semantically identical, keeps walrus happy."""
    for f in nc.m.functions:
        for bb in f.blocks:
            new_insts = []
            for inst in bb.instructions:
                si = inst.sync_info
                if si is not None and si.on_wait and len(si.on_wait) > MAX_WAITS:
                    waits = list(si.on_wait)
                    excess, keep = waits[MAX_WAITS:], waits[:MAX_WAITS]
                    for ci in range(0, len(excess), MAX_WAITS):
                        new_insts.append(mybir.InstNoOp(
                            name=f"{inst.name}-waitsplit{ci}",
                            engine=inst.engine, ins=[], outs=[],
                            sync_info=mybir.SyncInfo(
                                on_wait=excess[ci:ci + MAX_WAITS], on_update=[]),
                            text_hint="waitsplit"))
                    si.on_wait = keep
                new_insts.append(inst)
            bb.instructions = new_insts


def build_nc():
    nc = bass.Bass()
    xT = nc.declare_dram_parameter("xT", [DM, S], F32R, isOutput=False)
    wq = nc.declare_dram_parameter("wq", [NOB, P, NT, P], F32R, isOutput=False)
    wk = nc.declare_dram_parameter("wk", [NOB, P, NT, P], F32R, isOutput=False)
    wv = nc.declare_dram_parameter("wv", [NODT, P, NT, 512], BF16, isOutput=False)
    qb = nc.declare_dram_parameter("qb", [P, NOB], F32, isOutput=False)
    kb = nc.declare_dram_parameter("kb", [P, NOB], F32, isOutput=False)
    vb = nc.declare_dram_parameter("vb", [D2], BF16, isOutput=False)
    lam = nc.declare_dram_parameter("lam", [S], F32R, isOutput=False)
    ones = nc.declare_dram_parameter("ones", [P], F32R, isOutput=False)
    onesb = nc.declare_dram_parameter("onesb", [P], BF16, isOutput=False)
    xTb = nc.declare_dram_parameter("xTb", [DM, S], BF16, isOutput=False)
    out = nc.declare_dram_parameter("out", [S, D2], F32, isOutput=True)

    with tile.TileContext(nc) as tc, ExitStack() as top:
        const = top.enter_context(tc.tile_pool(name="const", bufs=1))
        xT_sb = const.tile([P, NT, S], F32R, name="xT_sb")
        xT_r = xT.rearrange("(t p) s -> p t s", p=P)
        for xc in range(4):
            nc.sync.dma_start(xT_sb[:, xc * (NT // 4):(xc + 1) * (NT // 4), :],
                              xT_r[:, xc * (NT // 4):(xc + 1) * (NT // 4), :])
        qb_sb = const.tile([P, NOB], F32, name="qb_sb")
        nc.sync.dma_start(qb_sb[:], qb[:])
        kb_sb = const.tile([P, NOB], F32, name="kb_sb")
        nc.sync.dma_start(kb_sb[:], kb[:])
        lam_sb = const.tile([1, S], F32R, name="lam_sb")
        nc.sync.dma_start(lam_sb[:], lam[None, :])
        ones_row = const.tile([1, P], F32R, name="ones_row")
        nc.sync.dma_start(ones_row[:], ones[None, :])
        onesb_row = const.tile([1, P], BF16, name="onesb_row")
        nc.sync.dma_start(onesb_row[:], onesb[None, :])
        xTb_sb = const.tile([P, NT, S], BF16, name="xTb_sb")
        ident = const.tile([P, P], F32, name="ident")
        make_identity(nc, ident[:])

        # lam broadcast to all 128 partitions via K=1 matmul
        lam_bc = const.tile([P, S], F32, name="lam_bc")
        with tc.tile_pool(name="ps_misc", bufs=1, space="PSUM") as ps_misc:
            pt = ps_misc.tile([P, 512], F32, name="lam_ps")
            nc.tensor.matmul(pt[:], ones_row[:], lam_sb[:], start=True, stop=True)
            nc.vector.tensor_copy(out=lam_bc[:], in_=pt[:])

        e_sb = const.tile([P, 2, NQB, S], F32, name="e_sb")
        d_sb = const.tile([P, 2, NQB], F32, name="d_sb")
        r_sb = const.tile([P, 2, NQB], F32, name="r_sb")
        P_sb = const.tile([P, NQB, S], F32, name="P_sb")
        PT_sb = const.tile([P, S // P, S], F32R, name="PT_sb")

        # ---- Phase A: fused q/k projections + score accumulation ----
        with ExitStack() as phA:
            wqk = phA.enter_context(tc.tile_pool(name="wqk", bufs=5))
            qksb = phA.enter_context(tc.tile_pool(name="qksb", bufs=6))
            ps_proj = phA.enter_context(tc.tile_pool(name="ps_proj", bufs=3, space="PSUM"))
            ps_scores = phA.enter_context(tc.tile_pool(name="ps_scores", bufs=5, space="PSUM"))

            for h in range(2):
                sc_tiles = [ps_scores.tile([P, S], F32, name=f"sc_{h}_{qbk}", tag="sc")
                            for qbk in range(NQB)]
                prev_qk = None

                def emit_scores(i, q_sb, k_sb):
                    for qbk in range(NQB):
                        nc.tensor.matmul(sc_tiles[qbk][:],
                                         q_sb[:, qbk * P:(qbk + 1) * P], k_sb[:],
                                         start=(i == 0), stop=(i == HOB - 1))

                for i in range(HOB):
                    ob = h * HOB + i
                    if h == 1 and i == HOB - 8:
                        nc.sync.dma_start(xTb_sb[:], xTb.rearrange("(t p) s -> p t s", p=P))
                    pq = ps_proj.tile([P, S], F32, name="pq", tag="pp")
                    for cw in range(2):
                        wt = wqk.tile([P, NT // 2, P], F32R, name="wt_q", tag="w")
                        nc.sync.dma_start(wt[:], wq[ob][:, cw * (NT // 2):(cw + 1) * (NT // 2), :])
                        for tt in range(NT // 2):
                            t = cw * (NT // 2) + tt
                            nc.tensor.matmul(pq[:], wt[:, tt, :], xT_sb[:, t, :],
                                             start=(t == 0), stop=(t == NT - 1))
                    q_sb = qksb.tile([P, S], F32R, name="q_sb", tag="qk")
                    nc.vector.tensor_scalar(q_sb[:], pq[:], qb_sb[:, ob:ob + 1], SCALE,
                                            mybir.AluOpType.add, mybir.AluOpType.mult)
                    pk = ps_proj.tile([P, S], F32, name="pk", tag="pp")
                    for cw in range(2):
                        wtk = wqk.tile([P, NT // 2, P], F32R, name="wt_k", tag="w")
                        nc.sync.dma_start(wtk[:], wk[ob][:, cw * (NT // 2):(cw + 1) * (NT // 2), :])
                        for tt in range(NT // 2):
                            t = cw * (NT // 2) + tt
                            nc.tensor.matmul(pk[:], wtk[:, tt, :], xT_sb[:, t, :],
                                             start=(t == 0), stop=(t == NT - 1))
                    if prev_qk is not None:
                        emit_scores(i - 1, *prev_qk)
                    k_sb = qksb.tile([P, S], F32R, name="k_sb", tag="qk")
                    nc.vector.tensor_scalar(k_sb[:], pk[:], kb_sb[:, ob:ob + 1], None,
                                            mybir.AluOpType.add)
                    prev_qk = (q_sb, k_sb)
                emit_scores(HOB - 1, *prev_qk)
                for qbk in range(NQB):
                    nc.scalar.activation(e_sb[:, h, qbk, :], sc_tiles[qbk][:],
                                         mybir.ActivationFunctionType.Exp,
                                         accum_out=d_sb[:, h, qbk:qbk + 1])

        # ---- Phase B+C: combine + transpose overlapped with v projection/AV ----
        nc.vector.reciprocal(r_sb[:, :, :], d_sb[:, :, :])
        with ExitStack() as phC:
            cmb = phC.enter_context(tc.tile_pool(name="cmb", bufs=2))
            wvp = phC.enter_context(tc.tile_pool(name="wvp", bufs=3))
            vbp = phC.enter_context(tc.tile_pool(name="vbp", bufs=2))
            vsb = phC.enter_context(tc.tile_pool(name="vsb", bufs=2))
            osb = phC.enter_context(tc.tile_pool(name="osb", bufs=4))
            ps_tr = phC.enter_context(tc.tile_pool(name="ps_tr", bufs=2, space="PSUM"))
            ps_vp = phC.enter_context(tc.tile_pool(name="ps_vp", bufs=4, space="PSUM"))
            ps_av = phC.enter_context(tc.tile_pool(name="ps_av", bufs=2, space="PSUM"))

            def combine_and_transpose():
                for qbk in range(NQB):
                    tmp = cmb.tile([P, S], F32, name="tmp", tag="tmp")
                    nc.vector.tensor_tensor(tmp[:], e_sb[:, 1, qbk, :], lam_bc[:],
                                            mybir.AluOpType.mult)
                    nc.vector.tensor_scalar(tmp[:], tmp[:], r_sb[:, 1, qbk:qbk + 1], None,
                                            mybir.AluOpType.mult)
                    nc.vector.tensor_scalar(P_sb[:, qbk, :], e_sb[:, 0, qbk, :],
                                            r_sb[:, 0, qbk:qbk + 1], None,
                                            mybir.AluOpType.mult)
                    nc.vector.tensor_tensor(P_sb[:, qbk, :], P_sb[:, qbk, :], tmp[:],
                                            mybir.AluOpType.subtract)
                for qbk in range(NQB):
                    for kbk in range(S // P):
                        pt2 = ps_tr.tile([P, P], F32, name="pt2", tag="pt")
                        nc.tensor.transpose(pt2[:], P_sb[:, qbk, kbk * P:(kbk + 1) * P],
                                            ident[:])
                        nc.vector.tensor_copy(out=PT_sb[:, kbk, qbk * P:(qbk + 1) * P],
                                              in_=pt2[:])

            v_tiles = [None] * NODT

            def do_av(odt):
                for qbk in range(NQB):
                    pav = ps_av.tile([P, 512], F32, name="pav", tag="av")
                    for kbk in range(S // P):
                        nc.tensor.matmul(pav[:],
                                         PT_sb[:, kbk, qbk * P:(qbk + 1) * P],
                                         v_tiles[odt][:, kbk, :],
                                         start=(kbk == 0), stop=(kbk == S // P - 1))
                    o_st = osb.tile([P, 512], F32, name="o_st", tag="o")
                    nc.vector.tensor_copy(out=o_st[:], in_=pav[:])
                    nc.sync.dma_start(out[qbk * P:(qbk + 1) * P, odt * 512:(odt + 1) * 512],
                                      o_st[:])

            for odt in range(NODT):
                pv = [ps_vp.tile([P, 512], F32, name=f"pv{sb}", tag="vp")
                      for sb in range(NQB)]
                vbt = vbp.tile([1, 512], BF16, name="vbt", tag="vb")
                nc.sync.dma_start(vbt[:], vb[None, odt * 512:(odt + 1) * 512])
                for sb in range(NQB):
                    nc.tensor.matmul(pv[sb][:], onesb_row[:], vbt[:],
                                     start=True, stop=False)
                for c in range(NCH):
                    wvt = wvp.tile([P, WV_CH, 512], BF16, name="wvt", tag="wv")
                    nc.sync.dma_start(wvt[:], wv[odt][:, c * WV_CH:(c + 1) * WV_CH, :])
                    for sb in range(NQB):
                        for tt in range(WV_CH):
                            t = c * WV_CH + tt
                            nc.tensor.matmul(pv[sb][:], xTb_sb[:, t, sb * P:(sb + 1) * P],
                                             wvt[:, tt, :],
                                             start=False, stop=(t == NT - 1))
                v_t = vsb.tile([P, S // P, 512], F32R, name="v_t", tag="v")
                for sb in range(NQB):
                    nc.vector.tensor_copy(out=v_t[:, sb, :], in_=pv[sb][:])
                v_tiles[odt] = v_t
                if odt == 0:
                    # PE chews on v-proj(0) while DVE does the combine and the
                    # transposes queue up behind it — hides the softmax tail.
                    combine_and_transpose()
                if odt >= 1:
                    do_av(odt - 1)
                    v_tiles[odt - 1] = None
            do_av(NODT - 1)

    _split_sync_waits(nc)
    return nc


def pack_shared(wq_w, wq_b, wk_w, wk_b, wv_w, wv_b,
                lambda_q1, lambda_k1, lambda_q2, lambda_k2):
    lam = (np.exp(lambda_q1 * lambda_k1) - np.exp(lambda_q2 * lambda_k2)
           + np.float32(0.8)).astype(np.float32)
    return {
        "wq": np.ascontiguousarray(wq_w.reshape(NOB, P, NT, P).transpose(0, 3, 2, 1)),
        "wk": np.ascontiguousarray(wk_w.reshape(NOB, P, NT, P).transpose(0, 3, 2, 1)),
        "wv": np.ascontiguousarray(wv_w.reshape(NODT, 512, NT, P).transpose(0, 3, 2, 1)).astype(ml_dtypes.bfloat16),
        "qb": np.ascontiguousarray(wq_b.reshape(NOB, P).T),
        "kb": np.ascontiguousarray(wk_b.reshape(NOB, P).T),
        "vb": np.ascontiguousarray(wv_b).astype(ml_dtypes.bfloat16),
        "lam": lam,
        "ones": np.ones(P, np.float32),
        "onesb": np.ones(P, ml_dtypes.bfloat16),
    }


def make_in_maps(x, wq_w, wq_b, wk_w, wk_b, wv_w, wv_b,
                 lambda_q1, lambda_k1, lambda_q2, lambda_k2):
    shared = pack_shared(wq_w, wq_b, wk_w, wk_b, wv_w, wv_b,
                         lambda_q1, lambda_k1, lambda_q2, lambda_k2)
    return [{**shared, "xT": np.ascontiguousarray(x[b].T),
         "xTb": np.ascontiguousarray(x[b].T).astype(ml_dtypes.bfloat16)} for b in range(B)]


_NC_CACHE = None


def get_nc():
    global _NC_CACHE
    if _NC_CACHE is None:
        _NC_CACHE = build_nc()
    return _NC_CACHE


def kernel(x, wq_w, wq_b, wk_w, wk_b, wv_w, wv_b,
           lambda_q1, lambda_k1, lambda_q2, lambda_k2):
    args = [np.asarray(a, dtype=np.float32) for a in
            (x, wq_w, wq_b, wk_w, wk_b, wv_w, wv_b,
             lambda_q1, lambda_k1, lambda_q2, lambda_k2)]
    nc = get_nc()
    in_maps = make_in_maps(*args)
    res = run_bass_kernel_spmd(nc, in_maps, list(range(B)))
    return np.stack([res.results[b]["out"] for b in range(B)]).astype(np.float32)



# revision 2
# speedup vs baseline: 1.4567x; 1.4567x over previous
"""Multi-head differential attention (full-width) on 8 Trainium2 NeuronCores.

Contract: kernel(**inputs) takes the FULL inputs of reference.setup_inputs()
and returns the FULL [8, 512, 8192] float32 output.

Strategy: pure data-parallel over batch — one batch element per NeuronCore.

Key algebraic reduction: softmax scores depend on q/k only through
  q_h k_hᵀ = (x Wq_hᵀ + bq_h)(x Wk_hᵀ + bk_h)ᵀ
           = x (Wq_hᵀ Wk_h) xᵀ + [row-const] + 1·c_hᵀ + const,
and softmax over keys is invariant to per-row constants, so with
M_h = Wq_hᵀ Wk_h precomputed host-side (weights only) and the per-key
vector c_h = x(Wk_hᵀ bq_h) + bq_h·bk_h, the device never forms q or k:
  scores_h = scale·(x M_h xᵀ + 1 c_hᵀ).
This halves the projection matmul work (one y = x@M_h per half instead of
separate q and k projections), a ~31% cut in total PE cycles.

Per core, a single fused Bass/Tile kernel computes:
  - yT = M_hᵀ xᵀ o-block by o-block (bf16 stationary M-tiles, bf16 moving x),
    with the score matmuls (f32r yT × f32r xT, full PE rate) fused into the
    same loop one iteration behind, accumulating s_h = y_h xᵀ in PSUM,
  - softmax via ScalarE Exp with accum_out row-sums (scores bounded ~±25,
    so no max-subtraction is needed in fp32),
  - differential combine P = e1/d1 - lam ⊙ e2/d2 on VectorE,
  - PE-transpose of P, then a fused v-projection + attention-V matmul loop
    that streams Wv (bf16) and writes each 512-column output tile as it
    completes.

Weights are repacked host-side (layout permutation + the M_h = Wq_hᵀ Wk_h
products + dtype casts) so every DMA lands with long contiguous
per-partition runs.
"""
import ml_dtypes
import numpy as np
from contextlib import ExitStack

import concourse.bass as bass
import concourse.mybir as mybir
import concourse.tile as tile
from concourse.bass_utils import run_bass_kernel_spmd
from concourse.masks import make_identity

F32 = mybir.dt.float32
F32R = mybir.dt.float32r
BF16 = mybir.dt.bfloat16
P = 128
B = 8
S = 512          # sequence length (= d_head for the lambda broadcast)
DM = 4096        # model dim (projection contraction)
DH = 4096        # width of each q/k half (attention contraction)
D2 = 2 * DH      # projection output width
NQB = S // P     # 4 query blocks
NT = DM // P     # 32 contraction tiles
NJB = DM // P    # 32 o-blocks of y per half (y is [S, DM])
NODT = D2 // 512  # 16 output column tiles
WV_CH = 8        # dq-tiles per streamed Wv chunk
NCH = NT // WV_CH
SCALE = float(1.0 / np.sqrt(512.0))

MAX_WAITS = 1  # this walrus build allows one sync-wait per instruction


def _split_sync_waits(nc):
    """Move excess per-instruction sync waits onto preceding no-ops (same
    engine, program order) — semantically identical, keeps walrus happy."""
    for f in nc.m.functions:
        for bb in f.blocks:
            new_insts = []
            for inst in bb.instructions:
                si = inst.sync_info
                if si is not None and si.on_wait and len(si.on_wait) > MAX_WAITS:
                    waits = list(si.on_wait)
                    excess, keep = waits[MAX_WAITS:], waits[:MAX_WAITS]
                    for ci in range(0, len(excess), MAX_WAITS):
                        new_insts.append(mybir.InstNoOp(
                            name=f"{inst.name}-waitsplit{ci}",
                            engine=inst.engine, ins=[], outs=[],
                            sync_info=mybir.SyncInfo(
                                on_wait=excess[ci:ci + MAX_WAITS], on_update=[]),
                            text_hint="waitsplit"))
                    si.on_wait = keep
                new_insts.append(inst)
            bb.instructions = new_insts


def build_nc():
    nc = bass.Bass()
    xT = nc.declare_dram_parameter("xT", [DM, S], F32R, isOutput=False)
    xTb = nc.declare_dram_parameter("xTb", [DM, S], BF16, isOutput=False)
    Mw = nc.declare_dram_parameter("Mw", [2 * NJB, P, NT, P], BF16, isOutput=False)
    wv = nc.declare_dram_parameter("wv", [NODT, P, NT, 512], BF16, isOutput=False)
    cvec = nc.declare_dram_parameter("cvec", [2, S], F32R, isOutput=False)
    vb = nc.declare_dram_parameter("vb", [D2], BF16, isOutput=False)
    lam = nc.declare_dram_parameter("lam", [S], F32R, isOutput=False)
    ones = nc.declare_dram_parameter("ones", [P], F32R, isOutput=False)
    onesb = nc.declare_dram_parameter("onesb", [P], BF16, isOutput=False)
    out = nc.declare_dram_parameter("out", [S, D2], F32, isOutput=True)

    with tile.TileContext(nc) as tc, ExitStack() as top:
        const = top.enter_context(tc.tile_pool(name="const", bufs=1))
        # xT (f32r, rhs of the score matmuls) and xTb (bf16, rhs of the
        # proj matmuls + lhsT of phase C) both stay resident. Chunked DMA
        # so the first proj matmuls can start before the tail lands.
        xT_sb = const.tile([P, NT, S], F32R, name="xT_sb")
        xT_r = xT.rearrange("(t p) s -> p t s", p=P)
        xTb_sb = const.tile([P, NT, S], BF16, name="xTb_sb")
        xTb_r = xTb.rearrange("(t p) s -> p t s", p=P)
        for xc in range(4):
            sl = slice(xc * (NT // 4), (xc + 1) * (NT // 4))
            nc.sync.dma_start(xTb_sb[:, sl, :], xTb_r[:, sl, :])
            nc.sync.dma_start(xT_sb[:, sl, :], xT_r[:, sl, :])
        cv_sb = const.tile([1, 2, S], F32R, name="cv_sb")
        nc.sync.dma_start(cv_sb[:], cvec[None, :, :])
        lam_sb = const.tile([1, S], F32R, name="lam_sb")
        nc.sync.dma_start(lam_sb[:], lam[None, :])
        ones_row = const.tile([1, P], F32R, name="ones_row")
        nc.sync.dma_start(ones_row[:], ones[None, :])
        onesb_row = const.tile([1, P], BF16, name="onesb_row")
        nc.sync.dma_start(onesb_row[:], onesb[None, :])
        ident = const.tile([P, P], F32, name="ident")
        make_identity(nc, ident[:])

        # lam broadcast to all 128 partitions via K=1 matmul
        lam_bc = const.tile([P, S], F32, name="lam_bc")
        with tc.tile_pool(name="ps_misc", bufs=1, space="PSUM") as ps_misc:
            pt = ps_misc.tile([P, 512], F32, name="lam_ps")
            nc.tensor.matmul(pt[:], ones_row[:], lam_sb[:], start=True, stop=True)
            nc.vector.tensor_copy(out=lam_bc[:], in_=pt[:])

        e_sb = const.tile([P, 2, NQB, S], F32, name="e_sb")
        d_sb = const.tile([P, 2, NQB], F32, name="d_sb")
        r_sb = const.tile([P, 2, NQB], F32, name="r_sb")
        P_sb = const.tile([P, NQB, S], F32, name="P_sb")
        PT_sb = const.tile([P, S // P, S], F32R, name="PT_sb")

        # ---- Phase A: fused y = x@M_h projection + score accumulation ----
        with ExitStack() as phA:
            mpool = phA.enter_context(tc.tile_pool(name="mpool", bufs=5))
            ysb = phA.enter_context(tc.tile_pool(name="ysb", bufs=6))
            ps_proj = phA.enter_context(tc.tile_pool(name="ps_proj", bufs=3, space="PSUM"))
            ps_scores = phA.enter_context(tc.tile_pool(name="ps_scores", bufs=5, space="PSUM"))

            for h in range(2):
                sc_tiles = [ps_scores.tile([P, S], F32, name=f"sc_{h}_{qbk}", tag="sc")
                            for qbk in range(NQB)]
                # per-key bias vector (zero for zero biases): sc = 1·c_hᵀ
                for qbk in range(NQB):
                    nc.tensor.matmul(sc_tiles[qbk][:], ones_row[:],
                                     cv_sb[:, h, :], start=True, stop=False)

                def emit_scores(jb, y_t):
                    for qbk in range(NQB):
                        nc.tensor.matmul(sc_tiles[qbk][:],
                                         y_t[:, qbk * P:(qbk + 1) * P],
                                         xT_sb[:, jb, :],
                                         start=False, stop=(jb == NJB - 1))

                prev = None
                for jb in range(NJB):
                    pq = ps_proj.tile([P, S], F32, name="pq", tag="pp")
                    for cw in range(2):
                        mt = mpool.tile([P, NT // 2, P], BF16, name="mt", tag="m")
                        nc.sync.dma_start(
                            mt[:], Mw[h * NJB + jb][:, cw * (NT // 2):(cw + 1) * (NT // 2), :])
                        for tt in range(NT // 2):
                            t = cw * (NT // 2) + tt
                            nc.tensor.matmul(pq[:], mt[:, tt, :], xTb_sb[:, t, :],
                                             start=(t == 0), stop=(t == NT - 1))
                    y_t = ysb.tile([P, S], F32R, name="y_t", tag="y")
                    nc.vector.tensor_copy(out=y_t[:], in_=pq[:])
                    if prev is not None:
                        emit_scores(*prev)
                    prev = (jb, y_t)
                emit_scores(*prev)
                for qbk in range(NQB):
                    nc.scalar.activation(e_sb[:, h, qbk, :], sc_tiles[qbk][:],
                                         mybir.ActivationFunctionType.Exp,
                                         accum_out=d_sb[:, h, qbk:qbk + 1])

        # ---- Phase B+C: combine + transpose overlapped with v projection/AV ----
        nc.vector.reciprocal(r_sb[:, :, :], d_sb[:, :, :])
        with ExitStack() as phC:
            cmb = phC.enter_context(tc.tile_pool(name="cmb", bufs=2))
            wvp = phC.enter_context(tc.tile_pool(name="wvp", bufs=3))
            vbp = phC.enter_context(tc.tile_pool(name="vbp", bufs=2))
            vsb = phC.enter_context(tc.tile_pool(name="vsb", bufs=2))
            osb = phC.enter_context(tc.tile_pool(name="osb", bufs=4))
            ps_tr = phC.enter_context(tc.tile_pool(name="ps_tr", bufs=2, space="PSUM"))
            ps_vp = phC.enter_context(tc.tile_pool(name="ps_vp", bufs=4, space="PSUM"))
            ps_av = phC.enter_context(tc.tile_pool(name="ps_av", bufs=2, space="PSUM"))

            def combine_and_transpose():
                for qbk in range(NQB):
                    tmp = cmb.tile([P, S], F32, name="tmp", tag="tmp")
                    nc.vector.tensor_tensor(tmp[:], e_sb[:, 1, qbk, :], lam_bc[:],
                                            mybir.AluOpType.mult)
                    nc.vector.tensor_scalar(tmp[:], tmp[:], r_sb[:, 1, qbk:qbk + 1], None,
                                            mybir.AluOpType.mult)
                    nc.vector.tensor_scalar(P_sb[:, qbk, :], e_sb[:, 0, qbk, :],
                                            r_sb[:, 0, qbk:qbk + 1], None,
                                            mybir.AluOpType.mult)
                    nc.vector.tensor_tensor(P_sb[:, qbk, :], P_sb[:, qbk, :], tmp[:],
                                            mybir.AluOpType.subtract)
                for qbk in range(NQB):
                    for kbk in range(S // P):
                        pt2 = ps_tr.tile([P, P], F32, name="pt2", tag="pt")
                        nc.tensor.transpose(pt2[:], P_sb[:, qbk, kbk * P:(kbk + 1) * P],
                                            ident[:])
                        nc.vector.tensor_copy(out=PT_sb[:, kbk, qbk * P:(qbk + 1) * P],
                                              in_=pt2[:])

            v_tiles = [None] * NODT

            def do_av(odt):
                for qbk in range(NQB):
                    pav = ps_av.tile([P, 512], F32, name="pav", tag="av")
                    for kbk in range(S // P):
                        nc.tensor.matmul(pav[:],
                                         PT_sb[:, kbk, qbk * P:(qbk + 1) * P],
                                         v_tiles[odt][:, kbk, :],
                                         start=(kbk == 0), stop=(kbk == S // P - 1))
                    o_st = osb.tile([P, 512], F32, name="o_st", tag="o")
                    nc.vector.tensor_copy(out=o_st[:], in_=pav[:])
                    nc.sync.dma_start(out[qbk * P:(qbk + 1) * P, odt * 512:(odt + 1) * 512],
                                      o_st[:])

            for odt in range(NODT):
                pv = [ps_vp.tile([P, 512], F32, name=f"pv{sb}", tag="vp")
                      for sb in range(NQB)]
                vbt = vbp.tile([1, 512], BF16, name="vbt", tag="vb")
                nc.sync.dma_start(vbt[:], vb[None, odt * 512:(odt + 1) * 512])
                for sb in range(NQB):
                    nc.tensor.matmul(pv[sb][:], onesb_row[:], vbt[:],
                                     start=True, stop=False)
                for c in range(NCH):
                    wvt = wvp.tile([P, WV_CH, 512], BF16, name="wvt", tag="wv")
                    nc.sync.dma_start(wvt[:], wv[odt][:, c * WV_CH:(c + 1) * WV_CH, :])
                    for sb in range(NQB):
                        for tt in range(WV_CH):
                            t = c * WV_CH + tt
                            nc.tensor.matmul(pv[sb][:], xTb_sb[:, t, sb * P:(sb + 1) * P],
                                             wvt[:, tt, :],
                                             start=False, stop=(t == NT - 1))
                v_t = vsb.tile([P, S // P, 512], F32R, name="v_t", tag="v")
                for sb in range(NQB):
                    nc.vector.tensor_copy(out=v_t[:, sb, :], in_=pv[sb][:])
                v_tiles[odt] = v_t
                if odt == 0:
                    # PE chews on v-proj(0) while DVE does the combine and the
                    # transposes queue up behind it — hides the softmax tail.
                    combine_and_transpose()
                if odt >= 1:
                    do_av(odt - 1)
                    v_tiles[odt - 1] = None
            do_av(NODT - 1)

    _split_sync_waits(nc)
    return nc


def pack_shared(wq_w, wq_b, wk_w, wk_b, wv_w, wv_b,
                lambda_q1, lambda_k1, lambda_q2, lambda_k2):
    lam = (np.exp(lambda_q1 * lambda_k1) - np.exp(lambda_q2 * lambda_k2)
           + np.float32(0.8)).astype(np.float32)
    # M_h = Wq_hᵀ Wk_h  [dm_in, dm_in], with the softmax scale folded in.
    # pack[jb, p, tb, c] = M[tb*128 + p, jb*128 + c]
    Ms = []
    for h in range(2):
        Mh = (wq_w[h * DH:(h + 1) * DH].T @ wk_w[h * DH:(h + 1) * DH]
              ) * np.float32(SCALE)
        Ms.append(Mh.reshape(NT, P, NJB, P).transpose(2, 1, 0, 3))
    Mw = np.ascontiguousarray(np.concatenate(Ms, axis=0)).astype(ml_dtypes.bfloat16)
    # per-key score bias direction: c_h = x @ g_h + bq_h·bk_h (computed
    # per batch in make_in_maps); g_h = Wk_hᵀ bq_h
    g = np.stack([wk_w[h * DH:(h + 1) * DH].T @ wq_b[h * DH:(h + 1) * DH]
                  for h in range(2)])                      # [2, DM]
    cconst = np.array([wq_b[h * DH:(h + 1) * DH] @ wk_b[h * DH:(h + 1) * DH]
                       for h in range(2)], np.float32)     # [2]
    return {
        "Mw": Mw,
        "wv": np.ascontiguousarray(
            wv_w.reshape(NODT, 512, NT, P).transpose(0, 3, 2, 1)).astype(ml_dtypes.bfloat16),
        "vb": np.ascontiguousarray(wv_b).astype(ml_dtypes.bfloat16),
        "lam": lam,
        "ones": np.ones(P, np.float32),
        "onesb": np.ones(P, ml_dtypes.bfloat16),
    }, g, cconst


def make_in_maps(x, wq_w, wq_b, wk_w, wk_b, wv_w, wv_b,
                 lambda_q1, lambda_k1, lambda_q2, lambda_k2):
    shared, g, cconst = pack_shared(wq_w, wq_b, wk_w, wk_b, wv_w, wv_b,
                                    lambda_q1, lambda_k1, lambda_q2, lambda_k2)
    maps = []
    for b in range(B):
        xb = np.ascontiguousarray(x[b])
        cv = (xb @ g.T + cconst[None, :]).T * np.float32(SCALE)   # [2, S]
        maps.append({**shared,
                     "xT": np.ascontiguousarray(xb.T),
                     "xTb": np.ascontiguousarray(xb.T).astype(ml_dtypes.bfloat16),
                     "cvec": np.ascontiguousarray(cv.astype(np.float32))})
    return maps


_NC_CACHE = None


def get_nc():
    global _NC_CACHE
    if _NC_CACHE is None:
        _NC_CACHE = build_nc()
    return _NC_CACHE


def kernel(x, wq_w, wq_b, wk_w, wk_b, wv_w, wv_b,
           lambda_q1, lambda_k1, lambda_q2, lambda_k2):
    args = [np.asarray(a, dtype=np.float32) for a in
            (x, wq_w, wq_b, wk_w, wk_b, wv_w, wv_b,
             lambda_q1, lambda_k1, lambda_q2, lambda_k2)]
    nc = get_nc()
    in_maps = make_in_maps(*args)
    res = run_bass_kernel_spmd(nc, in_maps, list(range(B)))
    return np.stack([res.results[b]["out"] for b in range(B)]).astype(np.float32)


# revision 9
# speedup vs baseline: 1.4833x; 1.0183x over previous
"""Multi-head differential attention (full-width) on 8 Trainium2 NeuronCores.

Contract: kernel(**inputs) takes the FULL inputs of reference.setup_inputs()
and returns the FULL [8, 512, 8192] float32 output.

Strategy: pure data-parallel over batch — one batch element per NeuronCore.

Key algebraic reduction: softmax scores depend on q/k only through
  q_h k_hᵀ = (x Wq_hᵀ + bq_h)(x Wk_hᵀ + bk_h)ᵀ
           = x (Wq_hᵀ Wk_h) xᵀ + [row-const] + 1·c_hᵀ + const,
and softmax over keys is invariant to per-row constants, so with
M_h = Wq_hᵀ Wk_h precomputed host-side (weights only) and the per-key
vector c_h = x(Wk_hᵀ bq_h) + bq_h·bk_h, the device never forms q or k:
  scores_h = scale·(x M_h xᵀ + 1 c_hᵀ).
This halves the projection matmul work (one y = x@M_h per half instead of
separate q and k projections), a ~31% cut in total PE cycles.

Per core, a single fused Bass/Tile kernel computes:
  - yT = M_hᵀ xᵀ o-block by o-block (bf16 stationary M-tiles, bf16 moving x),
    with the score matmuls (f32r yT × f32r xT, full PE rate) fused into the
    same loop one iteration behind, accumulating s_h = y_h xᵀ in PSUM,
  - softmax via ScalarE Exp with accum_out row-sums (scores bounded ~±25,
    so no max-subtraction is needed in fp32),
  - differential combine P = e1/d1 - lam ⊙ e2/d2 on VectorE,
  - PE-transpose of P, then a fused v-projection + attention-V matmul loop
    that streams Wv (bf16) and writes each 512-column output tile as it
    completes.

Weights are repacked host-side (layout permutation + the M_h = Wq_hᵀ Wk_h
products + dtype casts) so every DMA lands with long contiguous
per-partition runs.
"""
import ml_dtypes
import numpy as np
from contextlib import ExitStack

import concourse.bass as bass
import concourse.mybir as mybir
import concourse.tile as tile
from concourse.bass_utils import run_bass_kernel_spmd
from concourse.masks import make_identity

F32 = mybir.dt.float32
F32R = mybir.dt.float32r
BF16 = mybir.dt.bfloat16
P = 128
B = 8
S = 512          # sequence length (= d_head for the lambda broadcast)
DM = 4096        # model dim (projection contraction)
DH = 4096        # width of each q/k half (attention contraction)
D2 = 2 * DH      # projection output width
NQB = S // P     # 4 query blocks
NT = DM // P     # 32 contraction tiles
NJB = DM // P    # 32 o-blocks of y per half (y is [S, DM])
NODT = D2 // 512  # 16 output column tiles
WV_CH = 8        # dq-tiles per streamed Wv chunk
NCH = NT // WV_CH
SCALE = float(1.0 / np.sqrt(512.0))

MAX_WAITS = 1  # this walrus build allows one sync-wait per instruction


def _split_sync_waits(nc):
    """Move excess per-instruction sync waits onto preceding no-ops (same
    engine, program order) — semantically identical, keeps walrus happy."""
    for f in nc.m.functions:
        for bb in f.blocks:
            new_insts = []
            for inst in bb.instructions:
                si = inst.sync_info
                if si is not None and si.on_wait and len(si.on_wait) > MAX_WAITS:
                    waits = list(si.on_wait)
                    excess, keep = waits[MAX_WAITS:], waits[:MAX_WAITS]
                    for ci in range(0, len(excess), MAX_WAITS):
                        new_insts.append(mybir.InstNoOp(
                            name=f"{inst.name}-waitsplit{ci}",
                            engine=inst.engine, ins=[], outs=[],
                            sync_info=mybir.SyncInfo(
                                on_wait=excess[ci:ci + MAX_WAITS], on_update=[]),
                            text_hint="waitsplit"))
                    si.on_wait = keep
                new_insts.append(inst)
            bb.instructions = new_insts


def build_nc():
    nc = bass.Bass()
    xT = nc.declare_dram_parameter("xT", [DM, S], F32R, isOutput=False)
    xTb = nc.declare_dram_parameter("xTb", [DM, S], BF16, isOutput=False)
    Mw = nc.declare_dram_parameter("Mw", [2 * NJB, P, NT, P], BF16, isOutput=False)
    wv = nc.declare_dram_parameter("wv", [NODT, P, NT, 512], BF16, isOutput=False)
    cvec = nc.declare_dram_parameter("cvec", [2, S], F32R, isOutput=False)
    vb = nc.declare_dram_parameter("vb", [D2], BF16, isOutput=False)
    lam = nc.declare_dram_parameter("lam", [S], F32R, isOutput=False)
    ones = nc.declare_dram_parameter("ones", [P], F32R, isOutput=False)
    onesb = nc.declare_dram_parameter("onesb", [P], BF16, isOutput=False)
    out = nc.declare_dram_parameter("out", [S, D2], F32, isOutput=True)

    with tile.TileContext(nc) as tc, ExitStack() as top:
        const = top.enter_context(tc.tile_pool(name="const", bufs=1))
        # Tiny parameter DMAs first so the lam broadcast + score-bias
        # matmuls can run while the big x DMAs stream in.
        cv_sb = const.tile([1, 2, S], F32R, name="cv_sb")
        nc.sync.dma_start(cv_sb[:], cvec[None, :, :])
        lam_sb = const.tile([1, S], F32R, name="lam_sb")
        nc.sync.dma_start(lam_sb[:], lam[None, :])
        ones_row = const.tile([1, P], F32R, name="ones_row")
        nc.sync.dma_start(ones_row[:], ones[None, :])
        onesb_row = const.tile([1, P], BF16, name="onesb_row")
        nc.sync.dma_start(onesb_row[:], onesb[None, :])
        ident = const.tile([P, P], F32, name="ident")
        make_identity(nc, ident[:])
        # xT (f32r, rhs of the score matmuls) and xTb (bf16, rhs of the
        # proj matmuls + lhsT of phase C) both stay resident. Their chunk
        # DMAs are interleaved with the first M-tile DMAs inside phase A
        # (see _x_dma_sched) so the PE starts ~5us in instead of waiting
        # for the full 12.6MB to land.
        xT_sb = const.tile([P, NT, S], F32R, name="xT_sb")
        xT_r = xT.rearrange("(t p) s -> p t s", p=P)
        xTb_sb = const.tile([P, NT, S], BF16, name="xTb_sb")
        xTb_r = xTb.rearrange("(t p) s -> p t s", p=P)

        def emit_x_chunk(which, xc):
            sl = slice(xc * (NT // 4), (xc + 1) * (NT // 4))
            if which == "xTb":
                nc.sync.dma_start(xTb_sb[:, sl, :], xTb_r[:, sl, :])
            else:
                nc.sync.dma_start(xT_sb[:, sl, :], xT_r[:, sl, :])

        # (h, jb, cw) -> x chunk DMAs to emit just before that M chunk DMA.
        # Every chunk must be emitted before its first reader in program
        # order: cw=0 of jb0 reads xTb tiles 0-15 (chunks 0,1), cw=1 reads
        # tiles 16-31 (chunks 2,3); emit_scores(jb) reads xT tile jb and is
        # emitted during jb+1.
        _x_dma_sched = {
            (0, 0, 0): [("xTb", 0), ("xTb", 1)],
            (0, 0, 1): [("xTb", 2), ("xTb", 3)],
            (0, 1, 0): [("xT", 0)],
            (0, 4, 0): [("xT", 1)],
            (0, 8, 0): [("xT", 2)],
            (0, 12, 0): [("xT", 3)],
        }

        # lam broadcast to all 128 partitions via K=1 matmul
        lam_bc = const.tile([P, S], F32, name="lam_bc")
        with tc.tile_pool(name="ps_misc", bufs=1, space="PSUM") as ps_misc:
            pt = ps_misc.tile([P, 512], F32, name="lam_ps")
            nc.tensor.matmul(pt[:], ones_row[:], lam_sb[:], start=True, stop=True)
            nc.vector.tensor_copy(out=lam_bc[:], in_=pt[:])

        e_sb = const.tile([P, 2, NQB, S], F32, name="e_sb")
        d_sb = const.tile([P, 2, NQB], F32, name="d_sb")
        r_sb = const.tile([P, 2, NQB], F32, name="r_sb")
        P_sb = const.tile([P, NQB, S], F32, name="P_sb")
        PT_sb = const.tile([P, S // P, S], F32R, name="PT_sb")

        # wv/vb pools live at top level so the first wv chunk can be
        # prefetched during phase A's tail (phase C then starts stall-free).
        wvp = top.enter_context(tc.tile_pool(name="wvp", bufs=3))
        vbp = top.enter_context(tc.tile_pool(name="vbp", bufs=2))

        # ---- Phase A: fused y = x@M_h projection + score accumulation ----
        with ExitStack() as phA:
            mpool = phA.enter_context(tc.tile_pool(name="mpool", bufs=4))
            ysb = phA.enter_context(tc.tile_pool(name="ysb", bufs=5))
            ps_proj = phA.enter_context(tc.tile_pool(name="ps_proj", bufs=3, space="PSUM"))
            ps_scores = phA.enter_context(tc.tile_pool(name="ps_scores", bufs=5, space="PSUM"))

            for h in range(2):
                sc_tiles = [ps_scores.tile([P, S], F32, name=f"sc_{h}_{qbk}", tag="sc")
                            for qbk in range(NQB)]
                # per-key bias vector (zero for zero biases): sc = 1·c_hᵀ
                for qbk in range(NQB):
                    nc.tensor.matmul(sc_tiles[qbk][:], ones_row[:],
                                     cv_sb[:, h, :], start=True, stop=False)

                def emit_scores(jb, y_t):
                    for qbk in range(NQB):
                        nc.tensor.matmul(sc_tiles[qbk][:],
                                         y_t[:, qbk * P:(qbk + 1) * P],
                                         xT_sb[:, jb, :],
                                         start=False, stop=(jb == NJB - 1))

                prev = None
                for jb in range(NJB):
                    pq = ps_proj.tile([P, S], F32, name="pq", tag="pp")
                    for cw in range(2):
                        for xd in _x_dma_sched.get((h, jb, cw), ()):
                            emit_x_chunk(*xd)
                        mt = mpool.tile([P, NT // 2, P], BF16, name="mt", tag="m")
                        nc.sync.dma_start(
                            mt[:], Mw[h * NJB + jb][:, cw * (NT // 2):(cw + 1) * (NT // 2), :])
                        for tt in range(NT // 2):
                            t = cw * (NT // 2) + tt
                            nc.tensor.matmul(pq[:], mt[:, tt, :], xTb_sb[:, t, :],
                                             start=(t == 0), stop=(t == NT - 1))
                    y_t = ysb.tile([P, S], F32R, name="y_t", tag="y")
                    nc.vector.tensor_copy(out=y_t[:], in_=pq[:])
                    if prev is not None:
                        emit_scores(*prev)
                    prev = (jb, y_t)
                emit_scores(*prev)
                for qbk in range(NQB):
                    nc.scalar.activation(e_sb[:, h, qbk, :], sc_tiles[qbk][:],
                                         mybir.ActivationFunctionType.Exp,
                                         accum_out=d_sb[:, h, qbk:qbk + 1])

            # prefetch phase C's first wv chunk + v-bias while the h=1
            # score tail still occupies the PE
            vbt0 = vbp.tile([1, 512], BF16, name="vbt0", tag="vb")
            nc.sync.dma_start(vbt0[:], vb[None, 0:512])
            wvt00 = wvp.tile([P, WV_CH, 512], BF16, name="wvt00", tag="wv")
            nc.sync.dma_start(wvt00[:], wv[0][:, 0:WV_CH, :])

        # ---- Phase B+C: combine + transpose overlapped with v projection/AV ----
        nc.vector.reciprocal(r_sb[:, :, :], d_sb[:, :, :])
        with ExitStack() as phC:
            cmb = phC.enter_context(tc.tile_pool(name="cmb", bufs=2))
            vsb = phC.enter_context(tc.tile_pool(name="vsb", bufs=2))
            osb = phC.enter_context(tc.tile_pool(name="osb", bufs=4))
            ps_tr = phC.enter_context(tc.tile_pool(name="ps_tr", bufs=2, space="PSUM"))
            ps_vp = phC.enter_context(tc.tile_pool(name="ps_vp", bufs=4, space="PSUM"))
            ps_av = phC.enter_context(tc.tile_pool(name="ps_av", bufs=2, space="PSUM"))

            def combine_and_transpose():
                for qbk in range(NQB):
                    tmp = cmb.tile([P, S], F32, name="tmp", tag="tmp")
                    nc.vector.tensor_tensor(tmp[:], e_sb[:, 1, qbk, :], lam_bc[:],
                                            mybir.AluOpType.mult)
                    nc.vector.tensor_scalar(tmp[:], tmp[:], r_sb[:, 1, qbk:qbk + 1], None,
                                            mybir.AluOpType.mult)
                    nc.vector.tensor_scalar(P_sb[:, qbk, :], e_sb[:, 0, qbk, :],
                                            r_sb[:, 0, qbk:qbk + 1], None,
                                            mybir.AluOpType.mult)
                    nc.vector.tensor_tensor(P_sb[:, qbk, :], P_sb[:, qbk, :], tmp[:],
                                            mybir.AluOpType.subtract)
                for qbk in range(NQB):
                    for kbk in range(S // P):
                        pt2 = ps_tr.tile([P, P], F32, name="pt2", tag="pt")
                        nc.tensor.transpose(pt2[:], P_sb[:, qbk, kbk * P:(kbk + 1) * P],
                                            ident[:])
                        nc.vector.tensor_copy(out=PT_sb[:, kbk, qbk * P:(qbk + 1) * P],
                                              in_=pt2[:])

            v_tiles = [None] * NODT

            def do_av(odt):
                for qbk in range(NQB):
                    pav = ps_av.tile([P, 512], F32, name="pav", tag="av")
                    for kbk in range(S // P):
                        nc.tensor.matmul(pav[:],
                                         PT_sb[:, kbk, qbk * P:(qbk + 1) * P],
                                         v_tiles[odt][:, kbk, :],
                                         start=(kbk == 0), stop=(kbk == S // P - 1))
                    o_st = osb.tile([P, 512], F32, name="o_st", tag="o")
                    nc.vector.tensor_copy(out=o_st[:], in_=pav[:])
                    nc.sync.dma_start(out[qbk * P:(qbk + 1) * P, odt * 512:(odt + 1) * 512],
                                      o_st[:])

            for odt in range(NODT):
                pv = [ps_vp.tile([P, 512], F32, name=f"pv{sb}", tag="vp")
                      for sb in range(NQB)]
                if odt == 0:
                    vbt = vbt0
                else:
                    vbt = vbp.tile([1, 512], BF16, name="vbt", tag="vb")
                    nc.sync.dma_start(vbt[:], vb[None, odt * 512:(odt + 1) * 512])
                for sb in range(NQB):
                    nc.tensor.matmul(pv[sb][:], onesb_row[:], vbt[:],
                                     start=True, stop=False)
                for c in range(NCH):
                    if odt == 0 and c == 0:
                        wvt = wvt00
                    else:
                        wvt = wvp.tile([P, WV_CH, 512], BF16, name="wvt", tag="wv")
                        nc.sync.dma_start(wvt[:], wv[odt][:, c * WV_CH:(c + 1) * WV_CH, :])
                    for sb in range(NQB):
                        for tt in range(WV_CH):
                            t = c * WV_CH + tt
                            nc.tensor.matmul(pv[sb][:], xTb_sb[:, t, sb * P:(sb + 1) * P],
                                             wvt[:, tt, :],
                                             start=False, stop=(t == NT - 1))
                v_t = vsb.tile([P, S // P, 512], F32R, name="v_t", tag="v")
                for sb in range(NQB):
                    nc.vector.tensor_copy(out=v_t[:, sb, :], in_=pv[sb][:])
                v_tiles[odt] = v_t
                if odt == 0:
                    # PE chews on v-proj(0) while DVE does the combine and the
                    # transposes queue up behind it — hides the softmax tail.
                    combine_and_transpose()
                if odt >= 1:
                    do_av(odt - 1)
                    v_tiles[odt - 1] = None
            do_av(NODT - 1)

    _split_sync_waits(nc)
    return nc


def pack_shared(wq_w, wq_b, wk_w, wk_b, wv_w, wv_b,
                lambda_q1, lambda_k1, lambda_q2, lambda_k2):
    lam = (np.exp(lambda_q1 * lambda_k1) - np.exp(lambda_q2 * lambda_k2)
           + np.float32(0.8)).astype(np.float32)
    # M_h = Wq_hᵀ Wk_h  [dm_in, dm_in], with the softmax scale folded in.
    # pack[jb, p, tb, c] = M[tb*128 + p, jb*128 + c]
    Ms = []
    for h in range(2):
        Mh = (wq_w[h * DH:(h + 1) * DH].T @ wk_w[h * DH:(h + 1) * DH]
              ) * np.float32(SCALE)
        Ms.append(Mh.reshape(NT, P, NJB, P).transpose(2, 1, 0, 3))
    Mw = np.ascontiguousarray(np.concatenate(Ms, axis=0)).astype(ml_dtypes.bfloat16)
    # per-key score bias direction: c_h = x @ g_h + bq_h·bk_h (computed
    # per batch in make_in_maps); g_h = Wk_hᵀ bq_h
    g = np.stack([wk_w[h * DH:(h + 1) * DH].T @ wq_b[h * DH:(h + 1) * DH]
                  for h in range(2)])                      # [2, DM]
    cconst = np.array([wq_b[h * DH:(h + 1) * DH] @ wk_b[h * DH:(h + 1) * DH]
                       for h in range(2)], np.float32)     # [2]
    return {
        "Mw": Mw,
        "wv": np.ascontiguousarray(
            wv_w.reshape(NODT, 512, NT, P).transpose(0, 3, 2, 1)).astype(ml_dtypes.bfloat16),
        "vb": np.ascontiguousarray(wv_b).astype(ml_dtypes.bfloat16),
        "lam": lam,
        "ones": np.ones(P, np.float32),
        "onesb": np.ones(P, ml_dtypes.bfloat16),
    }, g, cconst


def make_in_maps(x, wq_w, wq_b, wk_w, wk_b, wv_w, wv_b,
                 lambda_q1, lambda_k1, lambda_q2, lambda_k2):
    shared, g, cconst = pack_shared(wq_w, wq_b, wk_w, wk_b, wv_w, wv_b,
                                    lambda_q1, lambda_k1, lambda_q2, lambda_k2)
    maps = []
    for b in range(B):
        xb = np.ascontiguousarray(x[b])
        cv = (xb @ g.T + cconst[None, :]).T * np.float32(SCALE)   # [2, S]
        maps.append({**shared,
                     "xT": np.ascontiguousarray(xb.T),
                     "xTb": np.ascontiguousarray(xb.T).astype(ml_dtypes.bfloat16),
                     "cvec": np.ascontiguousarray(cv.astype(np.float32))})
    return maps


_NC_CACHE = None


def get_nc():
    global _NC_CACHE
    if _NC_CACHE is None:
        _NC_CACHE = build_nc()
    return _NC_CACHE


def kernel(x, wq_w, wq_b, wk_w, wk_b, wv_w, wv_b,
           lambda_q1, lambda_k1, lambda_q2, lambda_k2):
    args = [np.asarray(a, dtype=np.float32) for a in
            (x, wq_w, wq_b, wk_w, wk_b, wv_w, wv_b,
             lambda_q1, lambda_k1, lambda_q2, lambda_k2)]
    nc = get_nc()
    in_maps = make_in_maps(*args)
    res = run_bass_kernel_spmd(nc, in_maps, list(range(B)))
    return np.stack([res.results[b]["out"] for b in range(B)]).astype(np.float32)


# revision 14
# speedup vs baseline: 1.5438x; 1.0407x over previous
"""Multi-head differential attention (full-width) on 8 Trainium2 NeuronCores.

Contract: kernel(**inputs) takes the FULL inputs of reference.setup_inputs()
and returns the FULL [8, 512, 8192] float32 output.

Strategy: pure data-parallel over batch — one batch element per NeuronCore.

Algebraic reductions (all exact, modulo dtype rounding):
  1. Softmax scores depend on q/k only through
       q_h k_hᵀ = x (Wq_hᵀ Wk_h) xᵀ + [row-const] + 1·c_hᵀ + const
     and softmax over keys is invariant to per-row constants, so with
     M_h = Wq_hᵀ Wk_h precomputed host-side (weights only) and the per-key
     vector c_h = x(Wk_hᵀ bq_h) + bq_h·bk_h, the device never forms q or k:
       scores_h = scale·(x M_h xᵀ + 1 c_hᵀ).
     This halves the projection matmul work.
  2. The value path is reassociated: out = P (x Wvᵀ + 1 bvᵀ)
     = (Pᵀᵀ) … = Zᵀᵀ Wvᵀ + ρ bvᵀ with Z = xᵀ Pᵀ and ρ = rowsum(P), which
     replaces [v-projection + attention·V] by [Z-build + Z·Wvᵀ], saving the
     separate attention·V pass.
  3. Scores are accumulated directly in transposed [key, query] layout
     (lhsT = xT tile, rhs = y tile), so the softmax key-sums become
     ones-column matmuls, lam becomes a per-partition scalar, and no
     P-transposes are needed — Pᵀ is what the combine produces.

Per core, a single fused Bass/Tile kernel computes:
  - yT = M_hᵀ xᵀ o-block by o-block (bf16 stationary M-tiles, bf16 moving x),
    with the transposed score matmuls (f32r xT stationary × f32r y moving,
    full PE rate) fused into the same loop one iteration behind,
  - softmax via ScalarE Exp; key-sums via ones-column matmuls; reciprocal +
    K=1 broadcast matmul for the normalizers,
  - differential combine Pᵀ = e1ᵀ⊙r1 - lam_k ⊙ e2ᵀ⊙r2 on VectorE (bf16 out),
  - ZT = xᵀ Pᵀ (bf16), then out = Zᵀ Wvᵀ streaming Wv (bf16), writing each
    512-column output tile as it completes.

When the projection biases are zero (as in this workload) the bias terms
are compiled out; nonzero biases take an exact slow path (+~15us).
"""
import ml_dtypes
import numpy as np
from contextlib import ExitStack

import concourse.bass as bass
import concourse.mybir as mybir
import concourse.tile as tile
from concourse.bass_utils import run_bass_kernel_spmd

F32 = mybir.dt.float32
F32R = mybir.dt.float32r
BF16 = mybir.dt.bfloat16
P = 128
B = 8
S = 512          # sequence length
DM = 4096        # model dim (projection contraction)
DH = 4096        # width of each q/k half (attention contraction)
D2 = 2 * DH      # projection output width
NKB = S // P     # 4 key blocks
NT = DM // P     # 32 contraction tiles
NJB = DM // P    # 32 o-blocks of y per half (y is [S, DM])
NODT = D2 // 512  # 16 output column tiles
WV_CH = 4        # dm-tiles per streamed Wv chunk
NCH = NT // WV_CH
SCALE = float(1.0 / np.sqrt(512.0))

MAX_WAITS = 1  # this walrus build allows one sync-wait per instruction


def _split_sync_waits(nc):
    """Move excess per-instruction sync waits onto preceding no-ops (same
    engine, program order) — semantically identical, keeps walrus happy."""
    for f in nc.m.functions:
        for bb in f.blocks:
            new_insts = []
            for inst in bb.instructions:
                si = inst.sync_info
                if si is not None and si.on_wait and len(si.on_wait) > MAX_WAITS:
                    waits = list(si.on_wait)
                    excess, keep = waits[MAX_WAITS:], waits[:MAX_WAITS]
                    for ci in range(0, len(excess), MAX_WAITS):
                        new_insts.append(mybir.InstNoOp(
                            name=f"{inst.name}-waitsplit{ci}",
                            engine=inst.engine, ins=[], outs=[],
                            sync_info=mybir.SyncInfo(
                                on_wait=excess[ci:ci + MAX_WAITS], on_update=[]),
                            text_hint="waitsplit"))
                    si.on_wait = keep
                new_insts.append(inst)
            bb.instructions = new_insts


def build_nc(with_qk_bias, with_v_bias):
    nc = bass.Bass()
    xT = nc.declare_dram_parameter("xT", [DM, S], F32R, isOutput=False)
    xTb = nc.declare_dram_parameter("xTb", [DM, S], BF16, isOutput=False)
    xN = nc.declare_dram_parameter("xN", [S, DM], BF16, isOutput=False)
    Mw = nc.declare_dram_parameter("Mw", [2 * NJB, P, NT, P], BF16, isOutput=False)
    wv = nc.declare_dram_parameter("wv", [NODT, P, NT, 512], BF16, isOutput=False)
    cvec = nc.declare_dram_parameter("cvec", [2, S], F32R, isOutput=False)
    vb = nc.declare_dram_parameter("vb", [D2], BF16, isOutput=False)
    lamc = nc.declare_dram_parameter("lamc", [P, NKB], F32, isOutput=False)
    ones = nc.declare_dram_parameter("ones", [P], F32R, isOutput=False)
    onesS = nc.declare_dram_parameter("onesS", [S], F32R, isOutput=False)
    onesb = nc.declare_dram_parameter("onesb", [P], BF16, isOutput=False)
    out = nc.declare_dram_parameter("out", [S, D2], F32, isOutput=True)

    with tile.TileContext(nc) as tc, ExitStack() as top:
        const = top.enter_context(tc.tile_pool(name="const", bufs=1))
        # Tiny parameter DMAs first so early PE work never waits on them.
        lamc_sb = const.tile([P, NKB], F32, name="lamc_sb")
        nc.sync.dma_start(lamc_sb[:], lamc[:])
        ones_col = const.tile([P, 1], F32R, name="ones_col")
        nc.sync.dma_start(ones_col[:], ones[:, None])
        ones_row = const.tile([1, P], F32R, name="ones_row")
        nc.sync.dma_start(ones_row[:], ones[None, :])
        if with_qk_bias:
            onesS_row = const.tile([1, S], F32R, name="onesS_row")
            nc.sync.dma_start(onesS_row[:], onesS[None, :])
            cv_sb = const.tile([1, 2, S], F32R, name="cv_sb")
            nc.sync.dma_start(cv_sb[:], cvec[None, :, :])
        if with_v_bias:
            onesb_col = const.tile([P, 1], BF16, name="onesb_col")
            nc.sync.dma_start(onesb_col[:], onesb[:, None])

        e_sb = const.tile([P, 2, NKB, S], F32R, name="e_sb")
        PT_sb = const.tile([P, NKB, S], BF16, name="PT_sb")
        rbc = const.tile([P, 2, S], F32, name="rbc")
        rT_sb = const.tile([1, 2, S], F32R, name="rT_sb")
        rhoT_sb = const.tile([1, S], BF16, name="rhoT_sb") if with_v_bias else None
        # x natural layout (bf16), lhsT of the ZT build; streamed during
        # phase A's second half.
        xN_sb = const.tile([P, NKB, DM], BF16, name="xN_sb")
        xN_r = xN.rearrange("(k p) d -> p k d", p=P)

        # wv/vb pools live at top level so the first wv chunk can be
        # prefetched during phase A's tail (phase C then starts stall-free).
        wvp = top.enter_context(tc.tile_pool(name="wvp", bufs=3))
        vbp = top.enter_context(tc.tile_pool(name="vbp", bufs=2))

        # ---- Phase A: fused y = x@M_h projection + transposed scores ----
        with ExitStack() as phA:
            xpool = phA.enter_context(tc.tile_pool(name="xpool", bufs=1))
            # xT (f32r, stationary of the score matmuls) and xTb (bf16, rhs
            # of the proj matmuls) stay resident through phase A.
            xT_sb = xpool.tile([P, NT, S], F32R, name="xT_sb")
            xT_r = xT.rearrange("(t p) s -> p t s", p=P)
            xTb_sb = xpool.tile([P, NT, S], BF16, name="xTb_sb")
            xTb_r = xTb.rearrange("(t p) s -> p t s", p=P)

            def emit_x_chunk(which, xc):
                sl = slice(xc * (NT // 4), (xc + 1) * (NT // 4))
                if which == "xTb":
                    nc.sync.dma_start(xTb_sb[:, sl, :], xTb_r[:, sl, :])
                elif which == "xT":
                    nc.sync.dma_start(xT_sb[:, sl, :], xT_r[:, sl, :])
                else:
                    nc.sync.dma_start(xN_sb[:, xc, :], xN_r[:, xc, :])

            # (h, jb, cw) -> x chunk DMAs to emit just before that M chunk
            # DMA. Every chunk must be emitted before its first reader in
            # program order: cw=0 of jb0 reads xTb tiles 0-15 (chunks 0,1),
            # cw=1 reads tiles 16-31 (chunks 2,3); emit_scores(jb) reads xT
            # tile jb and is emitted during jb+1. xN is first read in
            # phase C — its chunks just smooth bandwidth during h=1.
            _x_dma_sched = {
                (0, 0, 0): [("xTb", 0), ("xTb", 1)],
                (0, 0, 1): [("xTb", 2), ("xTb", 3)],
                (0, 1, 0): [("xT", 0)],
                (0, 4, 0): [("xT", 1)],
                (0, 8, 0): [("xT", 2)],
                (0, 12, 0): [("xT", 3)],
                (1, 0, 0): [("xN", 0)],
                (1, 4, 0): [("xN", 1)],
                (1, 8, 0): [("xN", 2)],
                (1, 12, 0): [("xN", 3)],
            }

            mpool = phA.enter_context(tc.tile_pool(name="mpool", bufs=3))
            ysb = phA.enter_context(tc.tile_pool(name="ysb", bufs=4))
            ps_proj = phA.enter_context(tc.tile_pool(name="ps_proj", bufs=3, space="PSUM"))
            ps_scores = phA.enter_context(tc.tile_pool(name="ps_scores", bufs=4, space="PSUM"))

            for h in range(2):
                # sc_tiles[kbk] accumulates sTᵀ…: [128 keys, 512 queries]
                sc_tiles = [ps_scores.tile([P, S], F32, name=f"sc_{h}_{kbk}", tag="sc")
                            for kbk in range(NKB)]
                if with_qk_bias:
                    for kbk in range(NKB):
                        nc.tensor.matmul(sc_tiles[kbk][:],
                                         cv_sb[:, h, kbk * P:(kbk + 1) * P],
                                         onesS_row[:], start=True, stop=False)

                def emit_scores(jb, y_t):
                    for kbk in range(NKB):
                        nc.tensor.matmul(sc_tiles[kbk][:],
                                         xT_sb[:, jb, kbk * P:(kbk + 1) * P],
                                         y_t[:],
                                         start=(jb == 0 and not with_qk_bias),
                                         stop=(jb == NJB - 1))

                prev = None
                for jb in range(NJB):
                    pq = ps_proj.tile([P, S], F32, name="pq", tag="pp")
                    for cw in range(2):
                        for xd in _x_dma_sched.get((h, jb, cw), ()):
                            emit_x_chunk(*xd)
                        mt = mpool.tile([P, NT // 2, P], BF16, name="mt", tag="m")
                        nc.sync.dma_start(
                            mt[:], Mw[h * NJB + jb][:, cw * (NT // 2):(cw + 1) * (NT // 2), :])
                        for tt in range(NT // 2):
                            t = cw * (NT // 2) + tt
                            nc.tensor.matmul(pq[:], mt[:, tt, :], xTb_sb[:, t, :],
                                             start=(t == 0), stop=(t == NT - 1))
                    y_t = ysb.tile([P, S], F32R, name="y_t", tag="y")
                    nc.vector.tensor_copy(out=y_t[:], in_=pq[:])
                    if prev is not None:
                        emit_scores(*prev)
                    prev = (jb, y_t)
                emit_scores(*prev)
                # softmax pieces: exp, key-sums (ones-column matmul over the
                # partition axis), reciprocal, broadcast back to 128 rows
                for kbk in range(NKB):
                    nc.scalar.activation(e_sb[:, h, kbk, :], sc_tiles[kbk][:],
                                         mybir.ActivationFunctionType.Exp)
                dt_ps = ps_proj.tile([P, S], F32, name="dt_ps", tag="pp")
                for kbk in range(NKB):
                    nc.tensor.matmul(dt_ps[:1, :], ones_col[:], e_sb[:, h, kbk, :],
                                     start=(kbk == 0), stop=(kbk == NKB - 1))
                with nc.allow_low_precision(reason="f32r out, bit-identical to f32"):
                    nc.vector.reciprocal(rT_sb[:, h, :], dt_ps[:1, :])
                rb_ps = ps_proj.tile([P, S], F32, name="rb_ps", tag="pp")
                nc.tensor.matmul(rb_ps[:], ones_row[:], rT_sb[:, h, :],
                                 start=True, stop=True)
                nc.vector.tensor_copy(out=rbc[:, h, :], in_=rb_ps[:])

            # prefetch phase C's first wv chunk (+ v bias) while the h=1
            # softmax tail still occupies the PE
            if with_v_bias:
                vbt0 = vbp.tile([1, 512], BF16, name="vbt0", tag="vb")
                nc.sync.dma_start(vbt0[:], vb[None, 0:512])
            wvt00 = wvp.tile([P, WV_CH, 512], BF16, name="wvt00", tag="wv")
            nc.sync.dma_start(wvt00[:], wv[0][:, 0:WV_CH, :])

        # ---- Phase B: differential combine (transposed space) ----
        with ExitStack() as phC:
            cmb = phC.enter_context(tc.tile_pool(name="cmb", bufs=2))
            zsb = phC.enter_context(tc.tile_pool(name="zsb", bufs=1))
            osb = phC.enter_context(tc.tile_pool(name="osb", bufs=4))
            ps_zt = phC.enter_context(tc.tile_pool(name="ps_zt", bufs=2, space="PSUM"))
            ps_out = phC.enter_context(tc.tile_pool(name="ps_out", bufs=4, space="PSUM"))

            for kbk in range(NKB):
                tmp = cmb.tile([P, S], F32, name="tmp", tag="tmp")
                nc.vector.tensor_tensor(tmp[:], e_sb[:, 1, kbk, :], rbc[:, 1, :],
                                        mybir.AluOpType.mult)
                nc.vector.tensor_scalar(tmp[:], tmp[:], lamc_sb[:, kbk:kbk + 1], None,
                                        mybir.AluOpType.mult)
                t2 = cmb.tile([P, S], F32, name="t2", tag="t2")
                nc.vector.tensor_tensor(t2[:], e_sb[:, 0, kbk, :], rbc[:, 0, :],
                                        mybir.AluOpType.mult)
                with nc.allow_low_precision(reason="P in bf16 feeds bf16 matmuls"):
                    nc.vector.tensor_tensor(PT_sb[:, kbk, :], t2[:], tmp[:],
                                            mybir.AluOpType.subtract)

            if with_v_bias:
                rho_ps = ps_zt.tile([P, S], F32, name="rho_ps", tag="zt")
                for kbk in range(NKB):
                    nc.tensor.matmul(rho_ps[:1, :], onesb_col[:], PT_sb[:, kbk, :],
                                     start=(kbk == 0), stop=(kbk == NKB - 1))
                with nc.allow_low_precision(reason="rho in bf16 feeds bf16 matmuls"):
                    nc.vector.tensor_copy(out=rhoT_sb[:], in_=rho_ps[:1, :])

            # ---- ZT = xᵀ Pᵀ : [dm, q] in 32 o-blocks ----
            ZT_sb = zsb.tile([P, NT, S], BF16, name="ZT_sb")
            for dmb in range(NT):
                zt_ps = ps_zt.tile([P, S], F32, name="zt_ps", tag="zt")
                for kbk in range(NKB):
                    nc.tensor.matmul(zt_ps[:], xN_sb[:, kbk, dmb * P:(dmb + 1) * P],
                                     PT_sb[:, kbk, :],
                                     start=(kbk == 0), stop=(kbk == NKB - 1))
                with nc.allow_low_precision(reason="Z in bf16 feeds bf16 matmuls"):
                    nc.vector.tensor_copy(out=ZT_sb[:, dmb, :], in_=zt_ps[:])

            # ---- Phase C: out = Zᵀ Wvᵀ (+ ρ bvᵀ), streaming Wv ----
            for odt in range(NODT):
                po = [ps_out.tile([P, 512], F32, name=f"po{qbk}", tag="out")
                      for qbk in range(NKB)]
                if with_v_bias:
                    if odt == 0:
                        vbt = vbt0
                    else:
                        vbt = vbp.tile([1, 512], BF16, name="vbt", tag="vb")
                        nc.sync.dma_start(vbt[:], vb[None, odt * 512:(odt + 1) * 512])
                    for qbk in range(NKB):
                        nc.tensor.matmul(po[qbk][:],
                                         rhoT_sb[:, qbk * P:(qbk + 1) * P],
                                         vbt[:], start=True, stop=False)
                for c in range(NCH):
                    if odt == 0 and c == 0:
                        wvt = wvt00
                    else:
                        wvt = wvp.tile([P, WV_CH, 512], BF16, name="wvt", tag="wv")
                        nc.sync.dma_start(wvt[:], wv[odt][:, c * WV_CH:(c + 1) * WV_CH, :])
                    for qbk in range(NKB):
                        for tt in range(WV_CH):
                            t = c * WV_CH + tt
                            nc.tensor.matmul(po[qbk][:],
                                             ZT_sb[:, t, qbk * P:(qbk + 1) * P],
                                             wvt[:, tt, :],
                                             start=(t == 0 and not with_v_bias),
                                             stop=(t == NT - 1))
                for qbk in range(NKB):
                    o_st = osb.tile([P, 512], F32, name="o_st", tag="o")
                    nc.vector.tensor_copy(out=o_st[:], in_=po[qbk][:])
                    nc.sync.dma_start(out[qbk * P:(qbk + 1) * P, odt * 512:(odt + 1) * 512],
                                      o_st[:])

    _split_sync_waits(nc)
    return nc


def pack_shared(wq_w, wq_b, wk_w, wk_b, wv_w, wv_b,
                lambda_q1, lambda_k1, lambda_q2, lambda_k2):
    lam = (np.exp(lambda_q1 * lambda_k1) - np.exp(lambda_q2 * lambda_k2)
           + np.float32(0.8)).astype(np.float32)
    # M_h = Wq_hᵀ Wk_h  [dm_in, dm_in], with the softmax scale folded in.
    # pack[jb, p, tb, c] = M[tb*128 + p, jb*128 + c]
    Ms = []
    for h in range(2):
        Mh = (wq_w[h * DH:(h + 1) * DH].T @ wk_w[h * DH:(h + 1) * DH]
              ) * np.float32(SCALE)
        Ms.append(Mh.reshape(NT, P, NJB, P).transpose(2, 1, 0, 3))
    Mw = np.ascontiguousarray(np.concatenate(Ms, axis=0)).astype(ml_dtypes.bfloat16)
    # per-key score bias direction: c_h = x @ g_h + bq_h·bk_h (computed
    # per batch in make_in_maps); g_h = Wk_hᵀ bq_h
    g = np.stack([wk_w[h * DH:(h + 1) * DH].T @ wq_b[h * DH:(h + 1) * DH]
                  for h in range(2)])                      # [2, DM]
    cconst = np.array([wq_b[h * DH:(h + 1) * DH] @ wk_b[h * DH:(h + 1) * DH]
                       for h in range(2)], np.float32)     # [2]
    return {
        "Mw": Mw,
        "wv": np.ascontiguousarray(
            wv_w.reshape(NODT, 512, NT, P).transpose(0, 3, 2, 1)).astype(ml_dtypes.bfloat16),
        "vb": np.ascontiguousarray(wv_b).astype(ml_dtypes.bfloat16),
        "lamc": np.ascontiguousarray(lam.reshape(NKB, P).T),
        "ones": np.ones(P, np.float32),
        "onesS": np.ones(S, np.float32),
        "onesb": np.ones(P, ml_dtypes.bfloat16),
    }, g, cconst


def make_in_maps(x, wq_w, wq_b, wk_w, wk_b, wv_w, wv_b,
                 lambda_q1, lambda_k1, lambda_q2, lambda_k2):
    shared, g, cconst = pack_shared(wq_w, wq_b, wk_w, wk_b, wv_w, wv_b,
                                    lambda_q1, lambda_k1, lambda_q2, lambda_k2)
    maps = []
    for b in range(B):
        xb = np.ascontiguousarray(x[b])
        cv = (xb @ g.T + cconst[None, :]).T * np.float32(SCALE)   # [2, S]
        xbT = np.ascontiguousarray(xb.T)
        maps.append({**shared,
                     "xT": xbT,
                     "xTb": xbT.astype(ml_dtypes.bfloat16),
                     "xN": xb.astype(ml_dtypes.bfloat16),
                     "cvec": np.ascontiguousarray(cv.astype(np.float32))})
    return maps


_NC_CACHE = {}


def get_nc(with_qk_bias=False, with_v_bias=False):
    key = (with_qk_bias, with_v_bias)
    if key not in _NC_CACHE:
        _NC_CACHE[key] = build_nc(*key)
    return _NC_CACHE[key]


def kernel(x, wq_w, wq_b, wk_w, wk_b, wv_w, wv_b,
           lambda_q1, lambda_k1, lambda_q2, lambda_k2):
    args = [np.asarray(a, dtype=np.float32) for a in
            (x, wq_w, wq_b, wk_w, wk_b, wv_w, wv_b,
             lambda_q1, lambda_k1, lambda_q2, lambda_k2)]
    wq_b_, wk_b_, wv_b_ = args[2], args[4], args[6]
    nc = get_nc(bool(np.any(wq_b_) or np.any(wk_b_)), bool(np.any(wv_b_)))
    in_maps = make_in_maps(*args)
    res = run_bass_kernel_spmd(nc, in_maps, list(range(B)))
    return np.stack([res.results[b]["out"] for b in range(B)]).astype(np.float32)
